# revision 19
# baseline (speedup 1.0000x reference)
"""Trainium2 Bass kernel for the tanh-RNN language model.

Math (per reference):
    e      = embed[x]                                  [B,T,E]
    xproj  = e @ W_ih.T + b_ih + b_hh                  [B,T,H]
    h_t    = tanh(xproj_t + h_{t-1} @ W_hh.T)          scan over T
    out    = hs @ W_fc.T + b_fc                        [B,T,V]
    returns (out, h_T[None])

Device strategy (8 cores, data-parallel over batch, 32 seqs/core):
    Host precomputes table = embed @ W_ih.T + b_ih + b_hh  [V,H] and a
    one-hot matrix of x, so xproj arrives in PSUM via a single f32r
    matmul (table.T @ onehot) that prefills a whole PSUM bank (16 steps
    x 32 batch = 512 cols).  The serial recurrence is then, per step,
    one PE matmul (W_hh @ h accumulated onto the prefilled bank) and
    one ACT tanh (PSUM -> SBUF).  The output head (W_fc @ hs, f32r) and
    bias-add evacuation (DVE) run in the engines' idle windows.
"""

import os
import numpy as np

VOCAB, EMBED, HIDDEN = 27, 48, 128
NCORES = 8
BL = 32              # sequences per core
GROUP = 16           # steps per PSUM bank group (16*32 = 512 fp32 cols)
CH = 128             # steps per SBUF chunk (128*32 = 4096 cols)
GPC = CH // GROUP    # groups per chunk

LAST_EXEC_TIME_NS = None


def build(T):
    import concourse.bacc as bacc
    import concourse.mybir as mybir
    import concourse.tile as tile

    NCH = T // CH
    NG = T // GROUP
    f32 = mybir.dt.float32
    f32r = mybir.dt.float32r
    f16 = mybir.dt.float16
    Tanh = mybir.ActivationFunctionType.Tanh

    nc = bacc.Bacc("TRN2", target_bir_lowering=False, debug=False,
                   num_devices=NCORES)

    table_d = nc.dram_tensor("table", [VOCAB, HIDDEN], f32r, kind="ExternalInput")
    whhT_d = nc.dram_tensor("whhT", [HIDDEN, HIDDEN], f16, kind="ExternalInput")
    wfcT_d = nc.dram_tensor("wfcT", [HIDDEN, VOCAB], f16, kind="ExternalInput")
    bfc_d = nc.dram_tensor("bfc", [VOCAB, 1], f32, kind="ExternalInput")
    h0_d = nc.dram_tensor("h0", [HIDDEN, BL], f16, kind="ExternalInput")
    oh_d = [nc.dram_tensor(f"oh{c}", [VOCAB, CH * BL], f32r, kind="ExternalInput")
            for c in range(NCH)]
    out_d = [nc.dram_tensor(f"out{c}", [VOCAB, CH * BL], f32, kind="ExternalOutput")
             for c in range(NCH)]
    ht_d = nc.dram_tensor("ht", [HIDDEN, BL], f16, kind="ExternalOutput")

    with tile.TileContext(nc) as tc:
        with tc.sbuf_pool(name="sb", bufs=1) as sp, \
             tc.psum_pool(name="ps", bufs=1) as pp:
            table_sb = sp.tile([VOCAB, HIDDEN], f32r)
            whhT_sb = sp.tile([HIDDEN, HIDDEN], f16)
            wfcT_sb = sp.tile([HIDDEN, VOCAB], f16)
            bfc_sb = sp.tile([VOCAB, 1], f32)
            h0_sb = sp.tile([HIDDEN, BL], f16)
            oh_sb = [sp.tile([VOCAB, CH * BL], f32r, name=f"oh_sb{i}")
                     for i in range(2)]
            hs = [sp.tile([HIDDEN, CH * BL], f16, name=f"hs{i}")
                  for i in range(2)]
            osb = [sp.tile([VOCAB, CH * BL], f32, name=f"osb{i}")
                   for i in range(2)]
            rec = [pp.tile([HIDDEN, GROUP * BL], f32, name=f"rec{i}")
                   for i in range(4)]
            hps = [pp.tile([HIDDEN, GROUP * BL], f32, name=f"hps{i}")
                   for i in range(2)]

            dma = nc.sync.dma_start
            mm = nc.tensor.matmul
            act = nc.scalar.activation

            dma(table_sb[:], table_d[:])
            dma(whhT_sb[:], whhT_d[:])
            dma(wfcT_sb[:], wfcT_d[:])
            dma(bfc_sb[:], bfc_d[:])
            dma(h0_sb[:], h0_d[:])
            dma(oh_sb[0][:], oh_d[0][:])

            def prefill(g2, q):
                # eighth q (64 cols) of the one-hot xproj fill for bank g2.
                # Exactly one start=True per bank lifetime: start clears the
                # whole bank's PSUM written-bits, so only the first eighth may
                # set it; later eighths (start=False) replace fresh regions and
                # the chain then accumulates onto them.
                c2, gi = divmod(g2, GPC)
                col = gi * GROUP * BL + q * 64
                mm(rec[g2 % 4][:, q * 64:(q + 1) * 64],
                   lhsT=table_sb[:],
                   rhs=oh_sb[c2 % 2][:, col:col + 64],
                   start=(q == 0), stop=False, skip_group_check=True)

            def head(G, q):
                # quarter q of the [27, 512] head matmul for group G
                cG, gi = divmod(G, GPC)
                col = gi * GROUP * BL + q * (GROUP * BL // 4)
                w = GROUP * BL // 4
                mm(hps[G % 2][0:VOCAB, q * w:(q + 1) * w],
                   lhsT=wfcT_sb[:],
                   rhs=hs[cG % 2][:, col:col + w],
                   start=True, stop=True)

            def evac(G):
                cG, gi = divmod(G, GPC)
                col = gi * GROUP * BL
                nc.vector.tensor_scalar_add(
                    osb[cG % 2][0:VOCAB, col:col + GROUP * BL],
                    hps[G % 2][0:VOCAB, :],
                    bfc_sb[:, 0:1])

            for q in range(8):
                prefill(0, q)
            if NG > 1:
                for q in range(8):
                    prefill(1, q)

            PRE_SLOT = {0: 0, 1: 1, 2: 2, 3: 3, 4: 4, 6: 5, 8: 6, 10: 7}

            for t in range(T):
                g, gl = divmod(t, GROUP)
                c, tl = divmod(t, CH)
                if tl == 0 and c + 1 < NCH:
                    dma(oh_sb[(c + 1) % 2][:], oh_d[c + 1][:])
                if t == 0:
                    rhs_prev = h0_sb[:]
                else:
                    pc, ptl = divmod(t - 1, CH)
                    rhs_prev = hs[pc % 2][:, ptl * BL:(ptl + 1) * BL]
                mm(rec[g % 4][:, gl * BL:(gl + 1) * BL],
                   lhsT=whhT_sb[:], rhs=rhs_prev,
                   start=False, stop=(gl == GROUP - 1), skip_group_check=True)
                act(hs[c % 2][:, tl * BL:(tl + 1) * BL],
                    rec[g % 4][:, gl * BL:(gl + 1) * BL], Tanh)
                if gl in PRE_SLOT and g + 2 < NG:
                    prefill(g + 2, PRE_SLOT[gl])
                if gl in (5, 7, 9, 11) and g >= 1:
                    head(g - 1, (gl - 5) // 2)
                if gl == 13 and g >= 1:
                    evac(g - 1)
                if gl == 15 and g >= 1 and (g - 1) % GPC == GPC - 1:
                    cG = (g - 1) // GPC
                    dma(out_d[cG][:], osb[cG % 2][0:VOCAB, :])

            for q in range(4):
                head(NG - 1, q)
            evac(NG - 1)
            dma(out_d[NCH - 1][:], osb[(NCH - 1) % 2][0:VOCAB, :])
            dma(ht_d[:], hs[(NCH - 1) % 2][:, (CH - 1) * BL:CH * BL])

    nc.compile()
    return nc


def _prep_core(x_core, table, whhT, wfcT, bfc, T):
    NCH = T // CH
    idx = np.ascontiguousarray(x_core.T).reshape(NCH, CH * BL)
    k = np.arange(CH * BL)
    ins = {"table": table, "whhT": whhT, "wfcT": wfcT, "bfc": bfc,
           "h0": np.zeros((HIDDEN, BL), np.float16)}
    for c in range(NCH):
        oh = np.zeros((VOCAB, CH * BL), np.float32)
        oh[idx[c], k] = 1.0
        ins[f"oh{c}"] = oh
    return ins


def _maybe_install_trace_hook():
    """Self-contained copy of the axon NTFF profile hook (env-guarded)."""
    try:
        import contextlib
        import ctypes
        import sys
        import types

        lib = ctypes.CDLL("/opt/axon/libaxon_pjrt.so")
        if not hasattr(lib, "axon_start_nrt_profile"):
            return False
        lib.axon_start_nrt_profile.argtypes = [
            ctypes.POINTER(ctypes.c_int64), ctypes.c_size_t]
        lib.axon_start_nrt_profile.restype = ctypes.c_int64
        lib.axon_stop_nrt_profile.argtypes = [ctypes.c_char_p]
        lib.axon_stop_nrt_profile.restype = ctypes.c_int64

        @contextlib.contextmanager
        def _hook(output_dir, device_ids):
            import jax
            jax.devices()
            if device_ids:
                ids = (ctypes.c_int64 * len(device_ids))(*device_ids)
                rc = lib.axon_start_nrt_profile(ids, len(device_ids))
            else:
                rc = lib.axon_start_nrt_profile(None, 0)
            if rc != 0:
                raise RuntimeError(f"axon_start_nrt_profile rc={rc}")
            try:
                yield
            finally:
                n = lib.axon_stop_nrt_profile(str(output_dir).encode())
                print(f"ntff profile: {n} file(s) -> {output_dir}",
                      file=sys.stderr)

        mod = types.ModuleType("antenv.axon_hooks")
        mod.get_axon_ntff_profile_hook = lambda: _hook
        mod.set_axon_ntff_profile_hook = lambda h: None
        sys.modules["antenv.axon_hooks"] = mod

        from concourse import bass_utils
        bass_utils.upload_artifacts = lambda tmpdir: f"local:{tmpdir}"
        return True
    except Exception:
        return False


_NC_CACHE = {}


def kernel(**inputs):
    global LAST_EXEC_TIME_NS
    from concourse.bass_utils import run_bass_kernel_spmd

    x = np.asarray(inputs["x"]).astype(np.int32)
    embed = np.asarray(inputs["embed"], np.float32)
    W_ih = np.asarray(inputs["W_ih"], np.float32)
    W_hh = np.asarray(inputs["W_hh"], np.float32)
    b_ih = np.asarray(inputs["b_ih"], np.float32)
    b_hh = np.asarray(inputs["b_hh"], np.float32)
    W_fc = np.asarray(inputs["W_fc"], np.float32)
    b_fc = np.asarray(inputs["b_fc"], np.float32)

    B, T = x.shape
    assert B == NCORES * BL and T % CH == 0
    NCH = T // CH

    table = np.ascontiguousarray(embed @ W_ih.T + b_ih + b_hh)  # [V,H]
    whhT = np.ascontiguousarray(W_hh.T.astype(np.float16))
    wfcT = np.ascontiguousarray(W_fc.T.astype(np.float16))
    bfc = np.ascontiguousarray(b_fc.reshape(VOCAB, 1))

    if T not in _NC_CACHE:
        _NC_CACHE[T] = build(T)
    nc = _NC_CACHE[T]

    in_maps = [_prep_core(x[i * BL:(i + 1) * BL], table, whhT, wfcT, bfc, T)
               for i in range(NCORES)]

    trace = os.environ.get("KERNEL_TRACE", "") == "1"
    if trace:
        trace = _maybe_install_trace_hook()
    tmpdir = os.environ.get("KERNEL_TMPDIR") or None
    res = run_bass_kernel_spmd(nc, in_maps, core_ids=list(range(NCORES)),
                               trace=trace, tmpdir=tmpdir)
    LAST_EXEC_TIME_NS = getattr(res, "exec_time_ns", None)

    outs, hts = [], []
    for i in range(NCORES):
        r = res.results[i]
        chunks = [np.asarray(r[f"out{c}"]).reshape(VOCAB, CH, BL)
                  .transpose(2, 1, 0) for c in range(NCH)]
        outs.append(np.concatenate(chunks, axis=1))
        hts.append(np.asarray(r["ht"]).astype(np.float32).T)
    out = np.ascontiguousarray(np.concatenate(outs, axis=0), dtype=np.float32)
    hT = np.ascontiguousarray(np.concatenate(hts, axis=0), dtype=np.float32)[None]
    return out, hT


# revision 28
# speedup vs baseline: 1.0985x; 1.0985x over previous
"""Trainium2 Bass kernel for the tanh-RNN language model.

Math (per reference):
    e      = embed[x]                                  [B,T,E]
    xproj  = e @ W_ih.T + b_ih + b_hh                  [B,T,H]
    h_t    = tanh(xproj_t + h_{t-1} @ W_hh.T)          scan over T
    out    = hs @ W_fc.T + b_fc                        [B,T,V]
    returns (out, h_T[None])

Device strategy (8 cores, data-parallel over batch, 32 seqs/core):
    Host precomputes table = embed @ W_ih.T + b_ih + b_hh  [V,H] and a
    one-hot matrix of x, so xproj arrives in PSUM via a single f32r
    matmul (table.T @ onehot) that prefills a whole PSUM bank (16 steps
    x 32 batch = 512 cols).  The serial recurrence is then, per step,
    one PE matmul (W_hh @ h accumulated onto the prefilled bank) and
    one ACT tanh (PSUM -> SBUF).  The output head (W_fc @ hs, f32r) and
    bias-add evacuation (DVE) run in the engines' idle windows.
"""

import os
import numpy as np

VOCAB, EMBED, HIDDEN = 27, 48, 128
NCORES = 8
BL = 32              # sequences per core
GROUP = 16           # steps per PSUM bank group (16*32 = 512 fp32 cols)
CH = 128             # steps per SBUF chunk (128*32 = 4096 cols)
GPC = CH // GROUP    # groups per chunk

LAST_EXEC_TIME_NS = None


def build(T):
    import concourse.bacc as bacc
    import concourse.mybir as mybir
    import concourse.tile as tile

    NCH = T // CH
    NG = T // GROUP
    f32 = mybir.dt.float32
    f32r = mybir.dt.float32r
    f16 = mybir.dt.float16
    Tanh = mybir.ActivationFunctionType.Tanh

    nc = bacc.Bacc("TRN2", target_bir_lowering=False, debug=False,
                   num_devices=NCORES)

    table_d = nc.dram_tensor("table", [VOCAB, HIDDEN], f16, kind="ExternalInput")
    whhT_d = nc.dram_tensor("whhT", [HIDDEN, HIDDEN], f16, kind="ExternalInput")
    wfcT_d = nc.dram_tensor("wfcT", [HIDDEN, VOCAB], f16, kind="ExternalInput")
    bfc_d = nc.dram_tensor("bfc", [VOCAB, 1], f32, kind="ExternalInput")
    h0_d = nc.dram_tensor("h0", [HIDDEN, BL], f16, kind="ExternalInput")
    oh_d = [nc.dram_tensor(f"oh{c}", [VOCAB, CH * BL], f16, kind="ExternalInput")
            for c in range(NCH)]
    out_d = [nc.dram_tensor(f"out{c}", [VOCAB, CH * BL], f32, kind="ExternalOutput")
             for c in range(NCH)]
    ht_d = nc.dram_tensor("ht", [HIDDEN, BL], f16, kind="ExternalOutput")

    with tile.TileContext(nc) as tc:
        with tc.sbuf_pool(name="sb", bufs=1) as sp, \
             tc.psum_pool(name="ps", bufs=1) as pp:
            table_sb = sp.tile([VOCAB, HIDDEN], f16)
            whhT_sb = sp.tile([HIDDEN, HIDDEN], f16)
            wfcT_sb = sp.tile([HIDDEN, VOCAB], f16)
            bfc_sb = sp.tile([VOCAB, 1], f32)
            h0_sb = sp.tile([HIDDEN, BL], f16)
            oh_sb = [sp.tile([VOCAB, CH * BL], f16, name=f"oh_sb{i}")
                     for i in range(2)]
            hs = [sp.tile([HIDDEN, CH * BL], f16, name=f"hs{i}")
                  for i in range(2)]
            osb = [sp.tile([VOCAB, CH * BL], f32, name=f"osb{i}")
                   for i in range(2)]
            rec = [pp.tile([HIDDEN, GROUP * BL], f32, name=f"rec{i}")
                   for i in range(4)]
            hps = [pp.tile([HIDDEN, GROUP * BL], f32, name=f"hps{i}")
                   for i in range(2)]

            dma = nc.sync.dma_start
            mm = nc.tensor.matmul
            act = nc.scalar.activation

            dma(table_sb[:], table_d[:])
            dma(whhT_sb[:], whhT_d[:])
            dma(wfcT_sb[:], wfcT_d[:])
            dma(bfc_sb[:], bfc_d[:])
            dma(h0_sb[:], h0_d[:])
            dma(oh_sb[0][:], oh_d[0][:])

            def prefill(g2):
                c2, gi = divmod(g2, GPC)
                col = gi * GROUP * BL
                mm(rec[g2 % 4][:, :],
                   lhsT=table_sb[:],
                   rhs=oh_sb[c2 % 2][:, col:col + GROUP * BL],
                   start=True, stop=False, skip_group_check=True)

            def head(G, q):
                # quarter q of the [27, 512] head matmul for group G
                cG, gi = divmod(G, GPC)
                col = gi * GROUP * BL + q * (GROUP * BL // 4)
                w = GROUP * BL // 4
                mm(hps[G % 2][0:VOCAB, q * w:(q + 1) * w],
                   lhsT=wfcT_sb[:],
                   rhs=hs[cG % 2][:, col:col + w],
                   start=True, stop=True)

            def evac(G):
                cG, gi = divmod(G, GPC)
                col = gi * GROUP * BL
                nc.vector.tensor_scalar_add(
                    osb[cG % 2][0:VOCAB, col:col + GROUP * BL],
                    hps[G % 2][0:VOCAB, :],
                    bfc_sb[:, 0:1])

            prefill(0)
            if NG > 1:
                prefill(1)

            for t in range(T):
                g, gl = divmod(t, GROUP)
                c, tl = divmod(t, CH)
                if tl == 0 and c + 1 < NCH:
                    dma(oh_sb[(c + 1) % 2][:], oh_d[c + 1][:])
                if t == 0:
                    rhs_prev = h0_sb[:]
                else:
                    pc, ptl = divmod(t - 1, CH)
                    rhs_prev = hs[pc % 2][:, ptl * BL:(ptl + 1) * BL]
                mm(rec[g % 4][:, gl * BL:(gl + 1) * BL],
                   lhsT=whhT_sb[:], rhs=rhs_prev,
                   start=False, stop=(gl == GROUP - 1), skip_group_check=True)
                act(hs[c % 2][:, tl * BL:(tl + 1) * BL],
                    rec[g % 4][:, gl * BL:(gl + 1) * BL], Tanh)
                if gl == 0 and g + 2 < NG:
                    prefill(g + 2)
                if gl in (5, 7, 9, 11) and g >= 1:
                    head(g - 1, (gl - 5) // 2)
                if gl == 13 and g >= 1:
                    evac(g - 1)
                if gl == 15 and g >= 1 and (g - 1) % GPC == GPC - 1:
                    cG = (g - 1) // GPC
                    dma(out_d[cG][:], osb[cG % 2][0:VOCAB, :])

            for q in range(4):
                head(NG - 1, q)
            evac(NG - 1)
            dma(out_d[NCH - 1][:], osb[(NCH - 1) % 2][0:VOCAB, :])
            dma(ht_d[:], hs[(NCH - 1) % 2][:, (CH - 1) * BL:CH * BL])

    nc.compile()
    return nc


def _prep_core(x_core, table, whhT, wfcT, bfc, T):
    NCH = T // CH
    idx = np.ascontiguousarray(x_core.T).reshape(NCH, CH * BL)
    k = np.arange(CH * BL)
    ins = {"table": table, "whhT": whhT, "wfcT": wfcT, "bfc": bfc,
           "h0": np.zeros((HIDDEN, BL), np.float16)}
    for c in range(NCH):
        oh = np.zeros((VOCAB, CH * BL), np.float16)
        oh[idx[c], k] = 1.0
        ins[f"oh{c}"] = oh
    return ins


def _maybe_install_trace_hook():
    """Self-contained copy of the axon NTFF profile hook (env-guarded)."""
    try:
        import contextlib
        import ctypes
        import sys
        import types

        lib = ctypes.CDLL("/opt/axon/libaxon_pjrt.so")
        if not hasattr(lib, "axon_start_nrt_profile"):
            return False
        lib.axon_start_nrt_profile.argtypes = [
            ctypes.POINTER(ctypes.c_int64), ctypes.c_size_t]
        lib.axon_start_nrt_profile.restype = ctypes.c_int64
        lib.axon_stop_nrt_profile.argtypes = [ctypes.c_char_p]
        lib.axon_stop_nrt_profile.restype = ctypes.c_int64

        @contextlib.contextmanager
        def _hook(output_dir, device_ids):
            import jax
            jax.devices()
            if device_ids:
                ids = (ctypes.c_int64 * len(device_ids))(*device_ids)
                rc = lib.axon_start_nrt_profile(ids, len(device_ids))
            else:
                rc = lib.axon_start_nrt_profile(None, 0)
            if rc != 0:
                raise RuntimeError(f"axon_start_nrt_profile rc={rc}")
            try:
                yield
            finally:
                n = lib.axon_stop_nrt_profile(str(output_dir).encode())
                print(f"ntff profile: {n} file(s) -> {output_dir}",
                      file=sys.stderr)

        mod = types.ModuleType("antenv.axon_hooks")
        mod.get_axon_ntff_profile_hook = lambda: _hook
        mod.set_axon_ntff_profile_hook = lambda h: None
        sys.modules["antenv.axon_hooks"] = mod

        from concourse import bass_utils
        bass_utils.upload_artifacts = lambda tmpdir: f"local:{tmpdir}"
        return True
    except Exception:
        return False


_NC_CACHE = {}


def kernel(**inputs):
    global LAST_EXEC_TIME_NS
    from concourse.bass_utils import run_bass_kernel_spmd

    x = np.asarray(inputs["x"]).astype(np.int32)
    embed = np.asarray(inputs["embed"], np.float32)
    W_ih = np.asarray(inputs["W_ih"], np.float32)
    W_hh = np.asarray(inputs["W_hh"], np.float32)
    b_ih = np.asarray(inputs["b_ih"], np.float32)
    b_hh = np.asarray(inputs["b_hh"], np.float32)
    W_fc = np.asarray(inputs["W_fc"], np.float32)
    b_fc = np.asarray(inputs["b_fc"], np.float32)

    B, T = x.shape
    assert B == NCORES * BL and T % CH == 0
    NCH = T // CH

    table = np.ascontiguousarray(
        (embed @ W_ih.T + b_ih + b_hh).astype(np.float16))  # [V,H]
    whhT = np.ascontiguousarray(W_hh.T.astype(np.float16))
    wfcT = np.ascontiguousarray(W_fc.T.astype(np.float16))
    bfc = np.ascontiguousarray(b_fc.reshape(VOCAB, 1))

    if T not in _NC_CACHE:
        _NC_CACHE[T] = build(T)
    nc = _NC_CACHE[T]

    in_maps = [_prep_core(x[i * BL:(i + 1) * BL], table, whhT, wfcT, bfc, T)
               for i in range(NCORES)]

    trace = os.environ.get("KERNEL_TRACE", "") == "1"
    if trace:
        trace = _maybe_install_trace_hook()
    tmpdir = os.environ.get("KERNEL_TMPDIR") or None
    res = run_bass_kernel_spmd(nc, in_maps, core_ids=list(range(NCORES)),
                               trace=trace, tmpdir=tmpdir)
    LAST_EXEC_TIME_NS = getattr(res, "exec_time_ns", None)

    outs, hts = [], []
    for i in range(NCORES):
        r = res.results[i]
        chunks = [np.asarray(r[f"out{c}"]).reshape(VOCAB, CH, BL)
                  .transpose(2, 1, 0) for c in range(NCH)]
        outs.append(np.concatenate(chunks, axis=1))
        hts.append(np.asarray(r["ht"]).astype(np.float32).T)
    out = np.ascontiguousarray(np.concatenate(outs, axis=0), dtype=np.float32)
    hT = np.ascontiguousarray(np.concatenate(hts, axis=0), dtype=np.float32)[None]
    return out, hT
